# revision 23
# baseline (speedup 1.0000x reference)
"""CorrLookup Trainium2 kernel v2 (8 NeuronCores, SPMD data-parallel over pixels).

Reference op: for each pixel n (N = B*H*W = 16384) and each pyramid level l,
bilinear-sample an 81-point (9x9, radius 4) window centered at
(x_n + flow_x)/2^l from that pixel's own (H_l, W_l) correlation map, with
zero padding outside the map. Output (B, 4*81, H, W) f32.

v2 design (vs v1 baseline at 137us):
  - Merged gathers: ONE indirect DMA per level (2048 descriptors) instead of
    16 per level; SWDGE fixed overhead (~1us/inst) amortized. bf16 source
    data halves HBM gather bytes.
  - Host ships maps TRANSPOSED (x-major) with 12 zero guard COLUMNS each
    side: out-of-range x window positions read hosted zeros, so no x masks
    are needed on chip. Columns stay Hc tall; y handled by masks folded into
    the y-mix weights. Span per pixel-level stays the minimal 9*Hc+10.
  - Lookup centers are clamped on chip to [-5, Wc+4] (x) / [-5, Hc+4] (y),
    which provably preserves the zero-padded bilinear result while bounding
    the gather footprint.
  - x-first separable mix; engine split: GpSimd does gathers + level-0 mix
    (+ part of L3), DVE does the rest, ScalarE does 1-w.
"""

import os
import sys
import types
import numpy as np

B, H, W = 2, 64, 128
N = B * H * W
N_CORES = 8
NPX = N // N_CORES  # 2048
GPP = NPX // 128  # 16 pixels per partition
R = 4
LV = [(64, 128), (32, 64), (16, 32), (8, 16)]  # (Hc, Wc) per level
GX = 12  # zero guard columns each side
PAD = 1024  # element pad at buffer ends
SHIFT = 64.0  # coordinate shift so floor sees positive values
SPANS = [9 * Hc + 10 for Hc, _ in LV]
N_SWDGE_Q = int(os.environ.get("CORR_NSWQ", "1"))
LAST_EXEC_NS = None

_prog = None


def _install_trace_shim():
    try:
        import antenv

        if "antenv.axon_hooks" not in sys.modules:
            mod = types.ModuleType("antenv.axon_hooks")
            _h = [None]
            mod.set_axon_ntff_profile_hook = lambda hk: _h.__setitem__(0, hk)
            mod.get_axon_ntff_profile_hook = lambda: _h[0]
            sys.modules["antenv.axon_hooks"] = mod
            antenv.axon_hooks = mod
        from antenv.axon_hooks import set_axon_ntff_profile_hook

        from trn_agent_boot.trn_boot import _ntff_profile_via_ctypes

        set_axon_ntff_profile_hook(
            _ntff_profile_via_ctypes("/opt/axon/libaxon_pjrt.so")
        )
        import concourse.bass_utils as bu

        bu.upload_artifacts = lambda tmpdir: f"file://{tmpdir}"
        return True
    except Exception:
        return False


def _build():
    import concourse.bacc as bacc
    import concourse.bass as bass
    import concourse.tile as tile
    import concourse.mybir as mybir

    f32 = mybir.dt.float32
    bf16 = mybir.dt.bfloat16
    i32 = mybir.dt.int32
    Alu = mybir.AluOpType
    ActFn = mybir.ActivationFunctionType

    nc = bacc.Bacc(
        "TRN2",
        target_bir_lowering=False,
        debug=False,
        num_devices=N_CORES,
        num_swdge_queues=max(N_SWDGE_Q, 1),
        dynamic_dma_scratch_size=int(os.environ.get("CORR_DMASCRATCH", "16384")),
    )

    srcs = []
    for l, (Hc, Wc) in enumerate(LV):
        Wp = Wc + 2 * GX
        tot = NPX * Wp * Hc + 2 * PAD
        srcs.append(nc.dram_tensor(f"src{l}", [tot, 1], bf16, kind="ExternalInput").ap())
    # consts packed: [cxy(128) | hi2(128) | hct(128) | iot(10)]; cxy holds the
    # per-level scaled+shifted lookup centers, layout [l*32 + (x:0-15 | y:16-31)]
    cpack_in = nc.dram_tensor("cpack", [128, 394], f32, kind="ExternalInput").ap()
    base_in = nc.dram_tensor("baseH", [128, 64], i32, kind="ExternalInput").ap()
    outs = [
        nc.dram_tensor(f"out{l}", [128, GPP * 81], f32, kind="ExternalOutput").ap()
        for l in range(4)
    ]
    dbg = os.environ.get("CORR_DBG") == "1"
    if dbg:
        dbg_idx = nc.dram_tensor("dbg_idx", [128, 64], i32, kind="ExternalOutput").ap()
        dbg_p0 = nc.dram_tensor(
            "dbg_p0", [128, GPP * SPANS[0]], bf16, kind="ExternalOutput"
        ).ap()
        dbg_fxy = nc.dram_tensor("dbg_fxy", [128, 128], f32, kind="ExternalOutput").ap()
        dbg_cxy = nc.dram_tensor("dbg_cxy", [128, 128], f32, kind="ExternalOutput").ap()
        dbg_idxf = nc.dram_tensor("dbg_idxf", [128, 64], f32, kind="ExternalOutput").ap()

    def AP(tile_ap, off_extra, dims):
        base = tile_ap
        return bass.AP(base.tensor, base.offset + off_extra, [list(base.ap[0])] + dims)

    with tile.TileContext(nc) as tc:
        with (
            tc.tile_pool(name="const", bufs=1) as cp,
            tc.tile_pool(name="patch", bufs=1) as pp,
            tc.tile_pool(name="work", bufs=1) as wp,
        ):
            # ---- load constants (one packed DMA + baseH) ----
            cpack = cp.tile([128, 394], f32)
            baseH = cp.tile([128, 64], i32)
            nc.sync.dma_start(out=cpack[:], in_=cpack_in)
            nc.sync.dma_start(out=baseH[:], in_=base_in)
            cxy = cpack[:, 0:128]
            hi2 = cpack[:, 128:256]
            hct = cpack[:, 256:384]
            io_t = cpack[:, 384:394]

            # ---- clamp centers: cxy in [S-5, S+Wc+4] ----
            nc.vector.tensor_scalar(
                out=cxy, in0=cxy, scalar1=SHIFT - 5.0, scalar2=None, op0=Alu.max
            )
            nc.vector.tensor_tensor(out=cxy, in0=cxy, in1=hi2, op=Alu.min)

            # ---- floor + frac (rounding-mode independent) ----
            fi = wp.tile([128, 128], i32, tag="fi")
            ff = wp.tile([128, 128], f32, tag="ff")
            dd = wp.tile([128, 128], f32, tag="dd")
            mm = wp.tile([128, 128], f32, tag="mm")
            wxy = wp.tile([128, 128], bf16, tag="wxy")
            wxyf = wp.tile([128, 128], f32, tag="wxyf")
            fxy = wp.tile([128, 128], f32, tag="fxy")
            omxy = wp.tile([128, 128], bf16, tag="omxy")
            nc.vector.tensor_copy(out=fi[:], in_=cxy)
            nc.vector.tensor_copy(out=ff[:], in_=fi[:])
            nc.vector.tensor_tensor(out=dd[:], in0=cxy, in1=ff[:], op=Alu.subtract)
            nc.vector.tensor_scalar(
                out=mm[:], in0=dd[:], scalar1=0.0, scalar2=None, op0=Alu.is_lt
            )
            nc.vector.tensor_tensor(out=wxyf[:], in0=dd[:], in1=mm[:], op=Alu.add)
            nc.vector.tensor_copy(out=wxy[:], in_=wxyf[:])
            nc.vector.tensor_tensor(out=fxy[:], in0=ff[:], in1=mm[:], op=Alu.subtract)
            # 1 - w on the scalar engine (bf16 out)
            nc.scalar.activation(
                out=omxy[:], in_=wxyf[:], func=ActFn.Copy, bias=1.0, scale=-1.0
            )

            # ---- span start indices: idx = i32(fx*Hc + fy + .25) + baseH ----
            idxp = wp.tile([128, 128], f32, tag="idxp")
            idxf = wp.tile([128, 64], f32, tag="idxf")
            idxi = wp.tile([128, 64], i32, tag="idxi")
            idx = wp.tile([128, 64], i32, tag="idx")
            nc.vector.tensor_tensor(out=idxp[:], in0=fxy[:], in1=hct, op=Alu.mult)
            ix_v = AP(idxp[:], 0, [[32, 4], [1, 16]])
            iy_v = AP(idxp[:], 16, [[32, 4], [1, 16]])
            ox_v = AP(idxf[:], 0, [[16, 4], [1, 16]])
            nc.vector.tensor_tensor(out=ox_v, in0=ix_v, in1=iy_v, op=Alu.add)
            nc.vector.tensor_scalar(
                out=idxf[:], in0=idxf[:], scalar1=0.25, scalar2=None, op0=Alu.add
            )
            nc.vector.tensor_copy(out=idxi[:], in_=idxf[:])
            nc.vector.tensor_tensor(out=idx[:], in0=idxi[:], in1=baseH[:], op=Alu.add)

            # ---- merged gathers: one indirect DMA per level ----
            patches = []
            for l, (Hc, Wc) in enumerate(LV):
                span = SPANS[l]
                patch = pp.tile([128, GPP * span], bf16, tag=f"patch{l}")
                patches.append(patch)
            # HW SWDGE emits exactly one descriptor per partition per
            # instruction (streaming out-free-size contiguous elements from
            # the FIRST offset), so gathers must be per-(level, wave):
            # 64 instructions total.  Level 0 uses element_offset for its
            # upper half so on-chip idx stays < 2^24 (DVE int ALU is fp32
            # internally).
            gq = [0]

            def gather(l, g, elem_off=0):
                span = SPANS[l]
                bi = nc.gpsimd.indirect_dma_start(
                    out=patches[l][:, g * span : (g + 1) * span],
                    out_offset=None,
                    in_=srcs[l],
                    in_offset=bass.IndirectOffsetOnAxis(
                        ap=idx[:, l * 16 + g : l * 16 + g + 1], axis=0
                    ),
                    element_offset=elem_off,
                )
                # spread across SWDGE rings to avoid ring-wrap drains
                q = gq[0] % N_SWDGE_Q
                gq[0] += 1
                bi.ins.queue = f"qPoolDynamic{q or ''}"

            Hc0, Wc0 = LV[0]
            half0 = (NPX // 2) * (Wc0 + 2 * GX) * Hc0
            for g in range(GPP):
                gather(0, g, 0 if g < 8 else half0)
            for l in (1, 2, 3):
                for g in range(GPP):
                    gather(l, g)

            if dbg:
                nc.sync.dma_start(out=dbg_idx, in_=idx[:])
                nc.sync.dma_start(out=dbg_p0, in_=patches[0][:])
                nc.sync.dma_start(out=dbg_fxy, in_=fxy[:])
                nc.sync.dma_start(out=dbg_cxy, in_=cxy)
                nc.sync.dma_start(out=dbg_idxf, in_=idxf[:])

            # ---- y masks folded into y-mix weights ----
            # ys[l,g,r] = (fy - S) - 4 + r  (io hosts r-4-S)
            ys = wp.tile([128, 640], f32, tag="ys")
            ys_v = AP(ys[:], 0, [[160, 4], [10, GPP], [1, 10]])
            fy_b = AP(fxy[:], 16, [[32, 4], [1, GPP], [0, 10]])
            io_b = AP(io_t, 0, [[0, 4], [0, GPP], [1, 10]])
            nc.vector.tensor_tensor(out=ys_v, in0=fy_b, in1=io_b, op=Alu.add)
            mA = wp.tile([128, 640], f32, tag="mA")
            mB = wp.tile([128, 640], f32, tag="mB")
            nc.vector.tensor_scalar(
                out=mA[:], in0=ys[:], scalar1=0.0, scalar2=None, op0=Alu.is_ge
            )
            for l, (Hc, Wc) in enumerate(LV):
                nc.vector.tensor_scalar(
                    out=mB[:, l * 160 : (l + 1) * 160],
                    in0=ys[:, l * 160 : (l + 1) * 160],
                    scalar1=float(Hc - 1),
                    scalar2=None,
                    op0=Alu.is_le,
                )
            cmy = wp.tile([128, 640], bf16, tag="cmy")
            nc.vector.tensor_tensor(out=cmy[:], in0=mA[:], in1=mB[:], op=Alu.mult)
            # w0[l,g,b] = (1-wy)*cmy[b], w1[l,g,b] = wy*cmy[b+1]   (b=0..8)
            w0 = wp.tile([128, 4 * 144], bf16, tag="w0")
            w1 = wp.tile([128, 4 * 144], bf16, tag="w1")
            w0_v = AP(w0[:], 0, [[144, 4], [9, GPP], [1, 9]])
            w1_v = AP(w1[:], 0, [[144, 4], [9, GPP], [1, 9]])
            cmy0 = AP(cmy[:], 0, [[160, 4], [10, GPP], [1, 9]])
            cmy1 = AP(cmy[:], 1, [[160, 4], [10, GPP], [1, 9]])
            omy_b = AP(omxy[:], 16, [[32, 4], [1, GPP], [0, 9]])
            wy_b = AP(wxy[:], 16, [[32, 4], [1, GPP], [0, 9]])
            nc.vector.tensor_tensor(out=w0_v, in0=cmy0, in1=omy_b, op=Alu.mult)
            nc.vector.tensor_tensor(out=w1_v, in0=cmy1, in1=wy_b, op=Alu.mult)

            # ---- separable mix, x first ----
            # xm[g,a,b] = P[g,a,b]*(1-wx) + P[g,a+1,b]*wx   (a=0..8 cols, b=0..9 rows)
            # ot[g,a,b] = xm[g,a,b]*w0[b] + xm[g,a,b+1]*w1[b]
            t1 = wp.tile([128, GPP * 90], bf16, tag="t1")
            t2 = wp.tile([128, GPP * 90], bf16, tag="t2")
            xm = wp.tile([128, GPP * 90], bf16, tag="xm")
            u1 = wp.tile([128, GPP * 81], bf16, tag="u1")
            u2 = wp.tile([128, GPP * 81], bf16, tag="u2")

            def mix(l, eng, glo, ghi):
                Hc, Wc = LV[l]
                span = SPANS[l]
                patch = patches[l]
                G = ghi - glo
                P0 = AP(patch[:], glo * span, [[span, G], [Hc, 9], [1, 10]])
                P1 = AP(patch[:], glo * span + Hc, [[span, G], [Hc, 9], [1, 10]])
                t1v = AP(t1[:], 0, [[90, G], [10, 9], [1, 10]])
                t2v = AP(t2[:], 0, [[90, G], [10, 9], [1, 10]])
                omx_b = AP(omxy[:], l * 32 + glo, [[1, G], [0, 9], [0, 10]])
                wx_b = AP(wxy[:], l * 32 + glo, [[1, G], [0, 9], [0, 10]])
                eng.tensor_tensor(out=t1v, in0=P0, in1=omx_b, op=Alu.mult)
                eng.tensor_tensor(out=t2v, in0=P1, in1=wx_b, op=Alu.mult)
                xmf = AP(xm[:], 0, [[1, G * 90]])
                t1f = AP(t1[:], 0, [[1, G * 90]])
                t2f = AP(t2[:], 0, [[1, G * 90]])
                eng.tensor_tensor(out=xmf, in0=t1f, in1=t2f, op=Alu.add)
                xm0 = AP(xm[:], 0, [[90, G], [10, 9], [1, 9]])
                xm1 = AP(xm[:], 1, [[90, G], [10, 9], [1, 9]])
                u1v = AP(u1[:], 0, [[81, G], [9, 9], [1, 9]])
                u2v = AP(u2[:], 0, [[81, G], [9, 9], [1, 9]])
                w0_b = AP(w0[:], l * 144 + glo * 9, [[9, G], [0, 9], [1, 9]])
                w1_b = AP(w1[:], l * 144 + glo * 9, [[9, G], [0, 9], [1, 9]])
                eng.tensor_tensor(out=u1v, in0=xm0, in1=w0_b, op=Alu.mult)
                eng.tensor_tensor(out=u2v, in0=xm1, in1=w1_b, op=Alu.mult)
                ot = wp.tile([128, GPP * 81], f32, tag=f"ot{l % 2}")
                otv = AP(ot[:], glo * 81, [[1, G * 81]])
                u1f = AP(u1[:], 0, [[1, G * 81]])
                u2f = AP(u2[:], 0, [[1, G * 81]])
                eng.tensor_tensor(out=otv, in0=u1f, in1=u2f, op=Alu.add)
                out_slice = bass.AP(
                    outs[l].tensor,
                    outs[l].offset + glo * 81,
                    [list(outs[l].ap[0]), [1, G * 81]],
                )
                nc.sync.dma_start(out=out_slice, in_=otv)

            # All mixes on DVE (GpSimd is saturated by gather emission).
            # Last-gathered level (3) is mixed in quarters to shrink the tail.
            mix(0, nc.vector, 0, GPP)
            mix(1, nc.vector, 0, GPP)
            mix(2, nc.vector, 0, GPP)
            for q in range(4):
                mix(3, nc.vector, q * 4, (q + 1) * 4)

    nc.compile()
    return nc


def _marshal(corr0, corr1, corr2, corr3, flow):
    """Build per-core input maps."""
    import ml_dtypes

    bf16 = ml_dtypes.bfloat16
    corrs = [corr0, corr1, corr2, corr3]
    fl = np.ascontiguousarray(flow.transpose(0, 2, 3, 1).reshape(N, 2))
    wgrid = np.tile(np.arange(W, dtype=np.float32), H * B)
    hgrid = np.tile(np.repeat(np.arange(H, dtype=np.float32), W), B)
    gx_full = wgrid + fl[:, 0]
    gy_full = hgrid + fl[:, 1]
    iota = np.tile(
        (np.arange(10, dtype=np.float32) - 4.0 - SHIFT).reshape(1, 10), (128, 1)
    )

    # constants shared by all cores
    hi2 = np.empty((128, 128), dtype=np.float32)
    hct = np.empty((128, 128), dtype=np.float32)
    for l, (Hc, Wc) in enumerate(LV):
        hi2[:, l * 32 : l * 32 + 16] = SHIFT + Wc + 4.0
        hi2[:, l * 32 + 16 : l * 32 + 32] = SHIFT + Hc + 4.0
        hct[:, l * 32 : l * 32 + 16] = float(Hc)
        hct[:, l * 32 + 16 : l * 32 + 32] = 1.0

    def wm(a):  # [NPX] -> [128, GPP] with n_loc = g*128 + p
        return np.ascontiguousarray(a.reshape(GPP, 128).T)

    in_maps = []
    for c in range(N_CORES):
        m = {}
        lo = c * NPX
        cxy = np.empty((128, 128), dtype=np.float32)
        base = np.empty((128, 64), dtype=np.int32)
        n_loc = (np.arange(GPP)[None, :] * 128 + np.arange(128)[:, None]).astype(
            np.int64
        )
        for l, (Hc, Wc) in enumerate(LV):
            s = 1.0 / (1 << l)
            cxy[:, l * 32 : l * 32 + 16] = wm(gx_full[lo : lo + NPX]) * s + SHIFT
            cxy[:, l * 32 + 16 : l * 32 + 32] = wm(gy_full[lo : lo + NPX]) * s + SHIFT
            Wp = Wc + 2 * GX
            # level 0: idx relative to buffer half (element_offset in gather)
            n_rel = n_loc % (NPX // 2) if l == 0 else n_loc
            b = (
                PAD
                + n_rel * (Wp * Hc)
                + int((GX - 4 - SHIFT) * Hc)
                - 4
                - int(SHIFT)
            )
            base[:, l * 16 : (l + 1) * 16] = b.astype(np.int32)
            # maps: x-major with GX zero guard columns each side
            shard = corrs[l].reshape(N, Hc, Wc)[lo : lo + NPX]
            buf = np.zeros((NPX, Wp, Hc), dtype=bf16)
            buf[:, GX : GX + Wc, :] = shard.transpose(0, 2, 1).astype(bf16)
            full = np.zeros(NPX * Wp * Hc + 2 * PAD, dtype=bf16)
            full[PAD : PAD + NPX * Wp * Hc] = buf.reshape(-1)
            m[f"src{l}"] = full.reshape(-1, 1)
        cpk = np.empty((128, 394), dtype=np.float32)
        cpk[:, 0:128] = cxy
        cpk[:, 128:256] = hi2
        cpk[:, 256:384] = hct
        cpk[:, 384:394] = iota
        m["cpack"] = cpk
        m["baseH"] = base
        in_maps.append(m)
    return in_maps


def kernel(corr0, corr1, corr2, corr3, flow):
    global _prog, LAST_EXEC_NS
    trace = os.environ.get("CORR_TRACE") == "1"
    if trace:
        trace = _install_trace_shim()
    from concourse.bass_utils import run_bass_kernel_spmd

    if _prog is None:
        _prog = _build()
    in_maps = _marshal(corr0, corr1, corr2, corr3, flow)
    res = run_bass_kernel_spmd(
        _prog,
        in_maps,
        core_ids=list(range(N_CORES)),
        trace=trace,
        trace_cores=[0] if trace else None,
    )
    LAST_EXEC_NS = res.exec_time_ns
    if trace and res.instructions_and_trace:
        kernel.last_insts = res.instructions_and_trace
    full = np.empty((N, 324), dtype=np.float32)
    for c in range(N_CORES):
        lo = c * NPX
        for l in range(4):
            o = res.results[c][f"out{l}"].reshape(128, GPP, 81)
            full[lo : lo + NPX, l * 81 : (l + 1) * 81] = (
                o.transpose(1, 0, 2).reshape(NPX, 81)
            )
    return np.ascontiguousarray(
        full.reshape(B, H, W, 324).transpose(0, 3, 1, 2)
    )


# revision 24
# speedup vs baseline: 1.0067x; 1.0067x over previous
"""CorrLookup Trainium2 kernel v2 (8 NeuronCores, SPMD data-parallel over pixels).

Reference op: for each pixel n (N = B*H*W = 16384) and each pyramid level l,
bilinear-sample an 81-point (9x9, radius 4) window centered at
(x_n + flow_x)/2^l from that pixel's own (H_l, W_l) correlation map, with
zero padding outside the map. Output (B, 4*81, H, W) f32.

v2 design (vs v1 baseline at 137us):
  - Merged gathers: ONE indirect DMA per level (2048 descriptors) instead of
    16 per level; SWDGE fixed overhead (~1us/inst) amortized. bf16 source
    data halves HBM gather bytes.
  - Host ships maps TRANSPOSED (x-major) with 12 zero guard COLUMNS each
    side: out-of-range x window positions read hosted zeros, so no x masks
    are needed on chip. Columns stay Hc tall; y handled by masks folded into
    the y-mix weights. Span per pixel-level stays the minimal 9*Hc+10.
  - Lookup centers are clamped on chip to [-5, Wc+4] (x) / [-5, Hc+4] (y),
    which provably preserves the zero-padded bilinear result while bounding
    the gather footprint.
  - x-first separable mix; engine split: GpSimd does gathers + level-0 mix
    (+ part of L3), DVE does the rest, ScalarE does 1-w.
"""

import os
import sys
import types
import numpy as np

B, H, W = 2, 64, 128
N = B * H * W
N_CORES = 8
NPX = N // N_CORES  # 2048
GPP = NPX // 128  # 16 pixels per partition
R = 4
LV = [(64, 128), (32, 64), (16, 32), (8, 16)]  # (Hc, Wc) per level
GX = 12  # zero guard columns each side
PAD = 1024  # element pad at buffer ends
SHIFT = 64.0  # coordinate shift so floor sees positive values
SPANS = [9 * Hc + 10 for Hc, _ in LV]
N_SWDGE_Q = int(os.environ.get("CORR_NSWQ", "1"))
LAST_EXEC_NS = None

_prog = None


def _install_trace_shim():
    try:
        import antenv

        if "antenv.axon_hooks" not in sys.modules:
            mod = types.ModuleType("antenv.axon_hooks")
            _h = [None]
            mod.set_axon_ntff_profile_hook = lambda hk: _h.__setitem__(0, hk)
            mod.get_axon_ntff_profile_hook = lambda: _h[0]
            sys.modules["antenv.axon_hooks"] = mod
            antenv.axon_hooks = mod
        from antenv.axon_hooks import set_axon_ntff_profile_hook

        from trn_agent_boot.trn_boot import _ntff_profile_via_ctypes

        set_axon_ntff_profile_hook(
            _ntff_profile_via_ctypes("/opt/axon/libaxon_pjrt.so")
        )
        import concourse.bass_utils as bu

        bu.upload_artifacts = lambda tmpdir: f"file://{tmpdir}"
        return True
    except Exception:
        return False


def _build():
    import concourse.bacc as bacc
    import concourse.bass as bass
    import concourse.tile as tile
    import concourse.mybir as mybir

    f32 = mybir.dt.float32
    bf16 = mybir.dt.bfloat16
    i32 = mybir.dt.int32
    Alu = mybir.AluOpType
    ActFn = mybir.ActivationFunctionType

    nc = bacc.Bacc(
        "TRN2",
        target_bir_lowering=False,
        debug=False,
        num_devices=N_CORES,
        num_swdge_queues=max(N_SWDGE_Q, 1),
        dynamic_dma_scratch_size=int(os.environ.get("CORR_DMASCRATCH", "16384")),
    )

    srcs = []
    for l, (Hc, Wc) in enumerate(LV):
        Wp = Wc + 2 * GX
        tot = NPX * Wp * Hc + 2 * PAD
        srcs.append(nc.dram_tensor(f"src{l}", [tot, 1], bf16, kind="ExternalInput").ap())
    # consts packed: [cxy(128) | hi2(128) | hct(128) | iot(10)]; cxy holds the
    # per-level scaled+shifted lookup centers, layout [l*32 + (x:0-15 | y:16-31)]
    cpack_in = nc.dram_tensor("cpack", [128, 394], f32, kind="ExternalInput").ap()
    base_in = nc.dram_tensor("baseH", [128, 64], i32, kind="ExternalInput").ap()
    outs = [
        nc.dram_tensor(f"out{l}", [128, GPP * 81], f32, kind="ExternalOutput").ap()
        for l in range(4)
    ]
    dbg = os.environ.get("CORR_DBG") == "1"
    if dbg:
        dbg_idx = nc.dram_tensor("dbg_idx", [128, 64], i32, kind="ExternalOutput").ap()
        dbg_p0 = nc.dram_tensor(
            "dbg_p0", [128, GPP * SPANS[0]], bf16, kind="ExternalOutput"
        ).ap()
        dbg_fxy = nc.dram_tensor("dbg_fxy", [128, 128], f32, kind="ExternalOutput").ap()
        dbg_cxy = nc.dram_tensor("dbg_cxy", [128, 128], f32, kind="ExternalOutput").ap()
        dbg_idxf = nc.dram_tensor("dbg_idxf", [128, 64], f32, kind="ExternalOutput").ap()

    def AP(tile_ap, off_extra, dims):
        base = tile_ap
        return bass.AP(base.tensor, base.offset + off_extra, [list(base.ap[0])] + dims)

    with tile.TileContext(nc) as tc:
        with tc.tile_pool(name="main", bufs=1) as cp:
            pp = wp = cp
            # ---- load constants (one packed DMA + baseH) ----
            cpack = cp.tile([128, 394], f32)
            baseH = cp.tile([128, 64], i32)
            nc.sync.dma_start(out=cpack[:], in_=cpack_in)
            nc.sync.dma_start(out=baseH[:], in_=base_in)
            cxy = cpack[:, 0:128]
            hi2 = cpack[:, 128:256]
            hct = cpack[:, 256:384]
            io_t = cpack[:, 384:394]

            # ---- clamp centers: cxy in [S-5, S+Wc+4] ----
            nc.vector.tensor_scalar(
                out=cxy, in0=cxy, scalar1=SHIFT - 5.0, scalar2=None, op0=Alu.max
            )
            nc.vector.tensor_tensor(out=cxy, in0=cxy, in1=hi2, op=Alu.min)

            # ---- floor + frac (rounding-mode independent) ----
            fi = wp.tile([128, 128], i32, tag="fi")
            ff = wp.tile([128, 128], f32, tag="ff")
            dd = wp.tile([128, 128], f32, tag="dd")
            mm = wp.tile([128, 128], f32, tag="mm")
            wxy = wp.tile([128, 128], bf16, tag="wxy")
            wxyf = wp.tile([128, 128], f32, tag="wxyf")
            fxy = wp.tile([128, 128], f32, tag="fxy")
            omxy = wp.tile([128, 128], bf16, tag="omxy")
            nc.vector.tensor_copy(out=fi[:], in_=cxy)
            nc.vector.tensor_copy(out=ff[:], in_=fi[:])
            nc.vector.tensor_tensor(out=dd[:], in0=cxy, in1=ff[:], op=Alu.subtract)
            nc.vector.tensor_scalar(
                out=mm[:], in0=dd[:], scalar1=0.0, scalar2=None, op0=Alu.is_lt
            )
            nc.vector.tensor_tensor(out=wxyf[:], in0=dd[:], in1=mm[:], op=Alu.add)
            nc.vector.tensor_copy(out=wxy[:], in_=wxyf[:])
            nc.vector.tensor_tensor(out=fxy[:], in0=ff[:], in1=mm[:], op=Alu.subtract)
            # 1 - w on the scalar engine (bf16 out)
            nc.scalar.activation(
                out=omxy[:], in_=wxyf[:], func=ActFn.Copy, bias=1.0, scale=-1.0
            )

            # ---- span start indices: idx = i32(fx*Hc + fy + .25) + baseH ----
            idxp = wp.tile([128, 128], f32, tag="idxp")
            idxf = wp.tile([128, 64], f32, tag="idxf")
            idxi = wp.tile([128, 64], i32, tag="idxi")
            idx = wp.tile([128, 64], i32, tag="idx")
            nc.vector.tensor_tensor(out=idxp[:], in0=fxy[:], in1=hct, op=Alu.mult)
            ix_v = AP(idxp[:], 0, [[32, 4], [1, 16]])
            iy_v = AP(idxp[:], 16, [[32, 4], [1, 16]])
            ox_v = AP(idxf[:], 0, [[16, 4], [1, 16]])
            nc.vector.tensor_tensor(out=ox_v, in0=ix_v, in1=iy_v, op=Alu.add)
            nc.vector.tensor_scalar(
                out=idxf[:], in0=idxf[:], scalar1=0.25, scalar2=None, op0=Alu.add
            )
            nc.vector.tensor_copy(out=idxi[:], in_=idxf[:])
            nc.vector.tensor_tensor(out=idx[:], in0=idxi[:], in1=baseH[:], op=Alu.add)

            # ---- merged gathers: one indirect DMA per level ----
            patches = []
            for l, (Hc, Wc) in enumerate(LV):
                span = SPANS[l]
                patch = pp.tile([128, GPP * span], bf16, tag=f"patch{l}")
                patches.append(patch)
            # HW SWDGE emits exactly one descriptor per partition per
            # instruction (streaming out-free-size contiguous elements from
            # the FIRST offset), so gathers must be per-(level, wave):
            # 64 instructions total.  Level 0 uses element_offset for its
            # upper half so on-chip idx stays < 2^24 (DVE int ALU is fp32
            # internally).
            gq = [0]

            def gather(l, g, elem_off=0):
                span = SPANS[l]
                bi = nc.gpsimd.indirect_dma_start(
                    out=patches[l][:, g * span : (g + 1) * span],
                    out_offset=None,
                    in_=srcs[l],
                    in_offset=bass.IndirectOffsetOnAxis(
                        ap=idx[:, l * 16 + g : l * 16 + g + 1], axis=0
                    ),
                    element_offset=elem_off,
                )
                # spread across SWDGE rings to avoid ring-wrap drains
                q = gq[0] % N_SWDGE_Q
                gq[0] += 1
                bi.ins.queue = f"qPoolDynamic{q or ''}"

            Hc0, Wc0 = LV[0]
            half0 = (NPX // 2) * (Wc0 + 2 * GX) * Hc0
            for g in range(GPP):
                gather(0, g, 0 if g < 8 else half0)
            for l in (1, 2, 3):
                for g in range(GPP):
                    gather(l, g)

            if dbg:
                nc.sync.dma_start(out=dbg_idx, in_=idx[:])
                nc.sync.dma_start(out=dbg_p0, in_=patches[0][:])
                nc.sync.dma_start(out=dbg_fxy, in_=fxy[:])
                nc.sync.dma_start(out=dbg_cxy, in_=cxy)
                nc.sync.dma_start(out=dbg_idxf, in_=idxf[:])

            # ---- y masks folded into y-mix weights ----
            # ys[l,g,r] = (fy - S) - 4 + r  (io hosts r-4-S)
            ys = wp.tile([128, 640], f32, tag="ys")
            ys_v = AP(ys[:], 0, [[160, 4], [10, GPP], [1, 10]])
            fy_b = AP(fxy[:], 16, [[32, 4], [1, GPP], [0, 10]])
            io_b = AP(io_t, 0, [[0, 4], [0, GPP], [1, 10]])
            nc.vector.tensor_tensor(out=ys_v, in0=fy_b, in1=io_b, op=Alu.add)
            mA = wp.tile([128, 640], f32, tag="mA")
            mB = wp.tile([128, 640], f32, tag="mB")
            nc.vector.tensor_scalar(
                out=mA[:], in0=ys[:], scalar1=0.0, scalar2=None, op0=Alu.is_ge
            )
            for l, (Hc, Wc) in enumerate(LV):
                nc.vector.tensor_scalar(
                    out=mB[:, l * 160 : (l + 1) * 160],
                    in0=ys[:, l * 160 : (l + 1) * 160],
                    scalar1=float(Hc - 1),
                    scalar2=None,
                    op0=Alu.is_le,
                )
            cmy = wp.tile([128, 640], bf16, tag="cmy")
            nc.vector.tensor_tensor(out=cmy[:], in0=mA[:], in1=mB[:], op=Alu.mult)
            # w0[l,g,b] = (1-wy)*cmy[b], w1[l,g,b] = wy*cmy[b+1]   (b=0..8)
            w0 = wp.tile([128, 4 * 144], bf16, tag="w0")
            w1 = wp.tile([128, 4 * 144], bf16, tag="w1")
            w0_v = AP(w0[:], 0, [[144, 4], [9, GPP], [1, 9]])
            w1_v = AP(w1[:], 0, [[144, 4], [9, GPP], [1, 9]])
            cmy0 = AP(cmy[:], 0, [[160, 4], [10, GPP], [1, 9]])
            cmy1 = AP(cmy[:], 1, [[160, 4], [10, GPP], [1, 9]])
            omy_b = AP(omxy[:], 16, [[32, 4], [1, GPP], [0, 9]])
            wy_b = AP(wxy[:], 16, [[32, 4], [1, GPP], [0, 9]])
            nc.vector.tensor_tensor(out=w0_v, in0=cmy0, in1=omy_b, op=Alu.mult)
            nc.vector.tensor_tensor(out=w1_v, in0=cmy1, in1=wy_b, op=Alu.mult)

            # ---- separable mix, x first ----
            # xm[g,a,b] = P[g,a,b]*(1-wx) + P[g,a+1,b]*wx   (a=0..8 cols, b=0..9 rows)
            # ot[g,a,b] = xm[g,a,b]*w0[b] + xm[g,a,b+1]*w1[b]
            t1 = wp.tile([128, GPP * 90], bf16, tag="t1")
            t2 = wp.tile([128, GPP * 90], bf16, tag="t2")
            xm = wp.tile([128, GPP * 90], bf16, tag="xm")
            u1 = wp.tile([128, GPP * 81], bf16, tag="u1")
            u2 = wp.tile([128, GPP * 81], bf16, tag="u2")

            def mix(l, eng, glo, ghi):
                Hc, Wc = LV[l]
                span = SPANS[l]
                patch = patches[l]
                G = ghi - glo
                P0 = AP(patch[:], glo * span, [[span, G], [Hc, 9], [1, 10]])
                P1 = AP(patch[:], glo * span + Hc, [[span, G], [Hc, 9], [1, 10]])
                t1v = AP(t1[:], 0, [[90, G], [10, 9], [1, 10]])
                t2v = AP(t2[:], 0, [[90, G], [10, 9], [1, 10]])
                omx_b = AP(omxy[:], l * 32 + glo, [[1, G], [0, 9], [0, 10]])
                wx_b = AP(wxy[:], l * 32 + glo, [[1, G], [0, 9], [0, 10]])
                eng.tensor_tensor(out=t1v, in0=P0, in1=omx_b, op=Alu.mult)
                eng.tensor_tensor(out=t2v, in0=P1, in1=wx_b, op=Alu.mult)
                xmf = AP(xm[:], 0, [[1, G * 90]])
                t1f = AP(t1[:], 0, [[1, G * 90]])
                t2f = AP(t2[:], 0, [[1, G * 90]])
                eng.tensor_tensor(out=xmf, in0=t1f, in1=t2f, op=Alu.add)
                xm0 = AP(xm[:], 0, [[90, G], [10, 9], [1, 9]])
                xm1 = AP(xm[:], 1, [[90, G], [10, 9], [1, 9]])
                u1v = AP(u1[:], 0, [[81, G], [9, 9], [1, 9]])
                u2v = AP(u2[:], 0, [[81, G], [9, 9], [1, 9]])
                w0_b = AP(w0[:], l * 144 + glo * 9, [[9, G], [0, 9], [1, 9]])
                w1_b = AP(w1[:], l * 144 + glo * 9, [[9, G], [0, 9], [1, 9]])
                eng.tensor_tensor(out=u1v, in0=xm0, in1=w0_b, op=Alu.mult)
                eng.tensor_tensor(out=u2v, in0=xm1, in1=w1_b, op=Alu.mult)
                ot = wp.tile([128, GPP * 81], f32, tag=f"ot{l % 2}")
                otv = AP(ot[:], glo * 81, [[1, G * 81]])
                u1f = AP(u1[:], 0, [[1, G * 81]])
                u2f = AP(u2[:], 0, [[1, G * 81]])
                eng.tensor_tensor(out=otv, in0=u1f, in1=u2f, op=Alu.add)
                out_slice = bass.AP(
                    outs[l].tensor,
                    outs[l].offset + glo * 81,
                    [list(outs[l].ap[0]), [1, G * 81]],
                )
                nc.sync.dma_start(out=out_slice, in_=otv)

            # All mixes on DVE (GpSimd is saturated by gather emission).
            # Last-gathered level (3) is mixed in quarters to shrink the tail.
            mix(0, nc.vector, 0, GPP)
            mix(1, nc.vector, 0, GPP)
            mix(2, nc.vector, 0, GPP)
            for q in range(4):
                mix(3, nc.vector, q * 4, (q + 1) * 4)

    nc.compile()
    return nc


def _marshal(corr0, corr1, corr2, corr3, flow):
    """Build per-core input maps."""
    import ml_dtypes

    bf16 = ml_dtypes.bfloat16
    corrs = [corr0, corr1, corr2, corr3]
    fl = np.ascontiguousarray(flow.transpose(0, 2, 3, 1).reshape(N, 2))
    wgrid = np.tile(np.arange(W, dtype=np.float32), H * B)
    hgrid = np.tile(np.repeat(np.arange(H, dtype=np.float32), W), B)
    gx_full = wgrid + fl[:, 0]
    gy_full = hgrid + fl[:, 1]
    iota = np.tile(
        (np.arange(10, dtype=np.float32) - 4.0 - SHIFT).reshape(1, 10), (128, 1)
    )

    # constants shared by all cores
    hi2 = np.empty((128, 128), dtype=np.float32)
    hct = np.empty((128, 128), dtype=np.float32)
    for l, (Hc, Wc) in enumerate(LV):
        hi2[:, l * 32 : l * 32 + 16] = SHIFT + Wc + 4.0
        hi2[:, l * 32 + 16 : l * 32 + 32] = SHIFT + Hc + 4.0
        hct[:, l * 32 : l * 32 + 16] = float(Hc)
        hct[:, l * 32 + 16 : l * 32 + 32] = 1.0

    def wm(a):  # [NPX] -> [128, GPP] with n_loc = g*128 + p
        return np.ascontiguousarray(a.reshape(GPP, 128).T)

    in_maps = []
    for c in range(N_CORES):
        m = {}
        lo = c * NPX
        cxy = np.empty((128, 128), dtype=np.float32)
        base = np.empty((128, 64), dtype=np.int32)
        n_loc = (np.arange(GPP)[None, :] * 128 + np.arange(128)[:, None]).astype(
            np.int64
        )
        for l, (Hc, Wc) in enumerate(LV):
            s = 1.0 / (1 << l)
            cxy[:, l * 32 : l * 32 + 16] = wm(gx_full[lo : lo + NPX]) * s + SHIFT
            cxy[:, l * 32 + 16 : l * 32 + 32] = wm(gy_full[lo : lo + NPX]) * s + SHIFT
            Wp = Wc + 2 * GX
            # level 0: idx relative to buffer half (element_offset in gather)
            n_rel = n_loc % (NPX // 2) if l == 0 else n_loc
            b = (
                PAD
                + n_rel * (Wp * Hc)
                + int((GX - 4 - SHIFT) * Hc)
                - 4
                - int(SHIFT)
            )
            base[:, l * 16 : (l + 1) * 16] = b.astype(np.int32)
            # maps: x-major with GX zero guard columns each side
            shard = corrs[l].reshape(N, Hc, Wc)[lo : lo + NPX]
            buf = np.zeros((NPX, Wp, Hc), dtype=bf16)
            buf[:, GX : GX + Wc, :] = shard.transpose(0, 2, 1).astype(bf16)
            full = np.zeros(NPX * Wp * Hc + 2 * PAD, dtype=bf16)
            full[PAD : PAD + NPX * Wp * Hc] = buf.reshape(-1)
            m[f"src{l}"] = full.reshape(-1, 1)
        cpk = np.empty((128, 394), dtype=np.float32)
        cpk[:, 0:128] = cxy
        cpk[:, 128:256] = hi2
        cpk[:, 256:384] = hct
        cpk[:, 384:394] = iota
        m["cpack"] = cpk
        m["baseH"] = base
        in_maps.append(m)
    return in_maps


def kernel(corr0, corr1, corr2, corr3, flow):
    global _prog, LAST_EXEC_NS
    trace = os.environ.get("CORR_TRACE") == "1"
    if trace:
        trace = _install_trace_shim()
    from concourse.bass_utils import run_bass_kernel_spmd

    if _prog is None:
        _prog = _build()
    in_maps = _marshal(corr0, corr1, corr2, corr3, flow)
    res = run_bass_kernel_spmd(
        _prog,
        in_maps,
        core_ids=list(range(N_CORES)),
        trace=trace,
        trace_cores=[0] if trace else None,
    )
    LAST_EXEC_NS = res.exec_time_ns
    if trace and res.instructions_and_trace:
        kernel.last_insts = res.instructions_and_trace
    full = np.empty((N, 324), dtype=np.float32)
    for c in range(N_CORES):
        lo = c * NPX
        for l in range(4):
            o = res.results[c][f"out{l}"].reshape(128, GPP, 81)
            full[lo : lo + NPX, l * 81 : (l + 1) * 81] = (
                o.transpose(1, 0, 2).reshape(NPX, 81)
            )
    return np.ascontiguousarray(
        full.reshape(B, H, W, 324).transpose(0, 3, 1, 2)
    )
